# revision 21
# baseline (speedup 1.0000x reference)
"""DialogueGCN forward on 8 Trainium2 NeuronCores.

Strategy (expert-parallel over relations, per the sharding hint):
  The dominant cost is streaming the per-edge relation weights
  rgcn_weight[etype] (each [1024,1024]).  The dense 10x10 edge list uses
  only U <= 100 distinct relations out of R=200, so we gather just the
  used slices.  By linearity, the per-edge RGCN messages collapse to

      out = sum_r (A_r @ X) @ W_r        A_r[j,i] = mean-normalized
                                          attention weight of edge i->j
                                          with relation r (else 0)

  i.e. one matmul  [10, U*1024] x [U*1024, 1024].  We shard the U used
  relations across the 8 cores; each core computes its partial [10,1024]
  with a PSUM-accumulated K-loop on the tensor engine, and the partials
  are summed (all-reduce) on the host along with the tiny root/bias and
  second GraphConv layer terms (node features are [10,1024] - negligible).

  Device compute dtype is bf16 (rounded on host): halves the HBM traffic
  of the memory-bound weight stream; end-to-end L2 relative error vs the
  fp32 reference is ~5e-4.
"""

import os

import numpy as np

S = 10
N = 10
D = 1024
R = 2 * S * S
N_CORES = 8
KT = 128              # contraction tile rows (partition dim)
KT_PER_REL = D // KT  # 8 k-tiles per relation

# Filled with the slowest profiled core's HW time (ns) when profiling is
# available (it is not under the axon PJRT path - stays None there).
LAST_EXEC_TIME_NS = None

_NC_CACHE = {}


def _compute_dtype():
    import concourse.mybir as mybir
    import ml_dtypes

    name = os.environ.get("BASS_GCN_DTYPE", "bf16")
    if name == "bf16":
        return mybir.dt.bfloat16, ml_dtypes.bfloat16
    if name == "f32":
        return mybir.dt.float32, np.float32
    if name == "f32r":
        return mybir.dt.float32r, np.float32
    raise ValueError(name)


def _chunk_kt():
    return int(os.environ.get("BASS_GCN_CHUNK_KT", "4"))


def _wide_mm():
    return bool(int(os.environ.get("BASS_GCN_WIDE_MM", "0")))


def _build_nc(n_rel_local, dt_dev):
    """One core's program: out[10,1024] = sum_kt ht_kt.T @ wt_kt.

    Raw bass (no TileContext): the dependency structure is a straight
    pipeline, and Tile's kernel-tail drain needs one sync wait per live
    semaphore, which exceeds the walrus CTRL wait-slot limit once >6 DMA
    lanes are in flight.

    The weight stream is pre-tiled on the host into contiguous chunks of
    ch_kt k-tiles: wt[c] is [128, ch_kt*1024], k-tile g of the chunk in
    columns [g*1024:(g+1)*1024], partition p holding contraction row
    (c*ch_kt + g)*128 + p.  SP streams ht + all chunks on one HWDGE ring
    with a completion semaphore per chunk; PE chases the semaphores,
    accumulating every k-tile into PSUM; DVE copies PSUM to SBUF; SP
    DMAs the partial out.
    """
    import contextlib

    import concourse.bass as bass
    import concourse.mybir as mybir

    nkt = n_rel_local * KT_PER_REL   # total k-tiles
    ch_kt = min(_chunk_kt(), nkt)
    assert nkt % ch_kt == 0
    n_ch = nkt // ch_kt              # DMA chunks
    cfd = ch_kt * D                  # columns of one chunk
    wide = _wide_mm()

    nc = bass.Bass()
    ht = nc.declare_dram_parameter("ht", [KT, nkt * N], dt_dev, isOutput=False)
    wt = nc.declare_dram_parameter("wt", [n_ch, KT, cfd], dt_dev,
                                   isOutput=False)
    # 128 partitions (rows 10..127 are zeros): DMA completion semaphores
    # only fire all 16 increments for full-partition transfers.
    out = nc.declare_dram_parameter("out", [KT, D], mybir.dt.float32,
                                    isOutput=True)

    f32 = mybir.dt.float32
    with contextlib.ExitStack() as ctx:
        hsem = ctx.enter_context(nc.semaphore("hsem"))
        wsem = [
            ctx.enter_context(nc.semaphore(f"wsem{c}")) for c in range(n_ch)
        ]
        osem = ctx.enter_context(nc.semaphore("osem"))
        psem = ctx.enter_context(nc.semaphore("psem"))
        vsem = ctx.enter_context(nc.semaphore("vsem"))
        msem = ctx.enter_context(nc.semaphore("msem"))
        hts = ctx.enter_context(nc.sbuf_tensor("hts", [KT, nkt * N], dt_dev))
        ws = ctx.enter_context(nc.sbuf_tensor("ws", [KT, nkt * D], dt_dev))
        if wide:
            acc = ctx.enter_context(nc.psum_tensor("acc", [N, D], f32))
        else:
            acc0 = ctx.enter_context(nc.psum_tensor("acc0", [N, 512], f32))
            acc1 = ctx.enter_context(nc.psum_tensor("acc1", [N, 512], f32))
        o_s = ctx.enter_context(nc.sbuf_tensor("os", [KT, D], f32))

        with nc.Block() as block:

            @block.sync
            def _(sync):
                sync.dma_start(out=hts[:], in_=ht[:]).then_inc(hsem, 16)
                for c in range(n_ch):
                    sync.dma_start(
                        out=ws[:, c * cfd : (c + 1) * cfd], in_=wt[c]
                    ).then_inc(wsem[c], 16)
                sync.wait_ge(vsem, 1)
                sync.dma_start(out=out[:], in_=o_s[:]).then_inc(osem, 16)
                sync.wait_ge(osem, 16)

            @block.tensor
            def _(tensor):
                last_mm = None
                tensor.wait_ge(hsem, 16)
                for c in range(n_ch):
                    tensor.wait_ge(wsem[c], 16)
                    for g in range(ch_kt):
                        kt = c * ch_kt + g
                        lhsT = hts[:, kt * N : (kt + 1) * N]
                        first = kt == 0
                        last = kt == nkt - 1
                        col = kt * D
                        if wide:
                            last_mm = tensor.matmul(
                                acc[:], lhsT, ws[:, col : col + D],
                                start=first, stop=last,
                            )
                        else:
                            tensor.matmul(
                                acc0[:], lhsT, ws[:, col : col + 512],
                                start=first, stop=last,
                            )
                            last_mm = tensor.matmul(
                                acc1[:], lhsT, ws[:, col + 512 : col + D],
                                start=first, stop=last,
                            )
                last_mm.then_inc(psem, 1)

            @block.vector
            def _(vector):
                vector.memset(o_s[:, :], 0).then_inc(msem, 1)
                vector.wait_ge(msem, 1)
                vector.wait_ge(psem, 1)
                if wide:
                    vector.tensor_copy(o_s[0:N, :], acc[:]).then_inc(vsem, 1)
                else:
                    vector.tensor_copy(o_s[0:N, 0:512], acc0[:])
                    vector.tensor_copy(o_s[0:N, 512:1024],
                                       acc1[:]).then_inc(vsem, 1)

    return nc


def _prepare(global_features, speaker, Wq, Wk, rgcn_weight):
    """Host planning: attention, edge weights, per-relation aggregation,
    and the per-core shard arrays.  Returns (nc, batches, U) where
    batches is a list of in_maps (one per SPMD launch)."""
    x = np.asarray(global_features, dtype=np.float64)
    speaker = np.asarray(speaker)

    # ---- attention -> edge weights (tiny, host) ----
    q = x @ np.asarray(Wq, dtype=np.float64)
    k = x @ np.asarray(Wk, dtype=np.float64)
    logits = (q @ k.T) / np.sqrt(np.float64(D))
    logits -= logits.max(axis=-1, keepdims=True)
    attn = np.exp(logits)
    attn /= attn.sum(axis=-1, keepdims=True)

    # ---- dense all-pairs edges, relation ids, per-(rel,dst) mean weights ----
    src = np.repeat(np.arange(N), N)
    dst = np.tile(np.arange(N), N)
    sp = speaker.astype(np.int64)
    etype = 2 * (sp[src] * S + sp[dst]) + (src >= dst).astype(np.int64)
    used, inv = np.unique(etype, return_inverse=True)
    U = len(used)
    seg = etype * N + dst
    cnt = np.bincount(seg, minlength=R * N)
    w_e = attn.reshape(-1) / cnt[seg]

    # ---- per-used-relation aggregated pre-messages H[u,j,:] ----
    H = np.zeros((U, N, D))
    np.add.at(H, (inv, dst), w_e[:, None] * x[src])

    # ---- shard the used relations across cores (batched if U is huge) ----
    max_rel_local = int(os.environ.get("BASS_GCN_MAX_REL_LOCAL", "10"))
    n_rel_local = min(max_rel_local, max(1, -(-U // N_CORES)))
    rel_per_batch = N_CORES * n_rel_local
    n_batch = max(1, -(-U // rel_per_batch))

    dt_dev, dt_np = _compute_dtype()
    nkt = n_rel_local * KT_PER_REL
    ch_kt = min(_chunk_kt(), nkt)
    n_ch = nkt // ch_kt
    cfd = ch_kt * D

    key = (n_rel_local, ch_kt, _wide_mm(), str(dt_dev))
    if key not in _NC_CACHE:
        _NC_CACHE[key] = _build_nc(n_rel_local, dt_dev)
    nc = _NC_CACHE[key]

    rw = np.asarray(rgcn_weight)
    batches = []
    for b in range(n_batch):
        in_maps = []
        for c in range(N_CORES):
            lo = b * rel_per_batch + c * n_rel_local
            rel_ids = used[lo : lo + n_rel_local]
            n_real = len(rel_ids)

            # H shard -> ht layout [128, nkt*10]: k-tile kt occupies columns
            # [kt*10:(kt+1)*10]; partition p holds contraction row kt*128+p.
            h_c = np.zeros((n_rel_local, N, D))
            h_c[:n_real] = H[lo : lo + n_real]
            ht_h = (
                h_c.transpose(0, 2, 1)          # [n_rel, D, N]
                .reshape(nkt, KT, N)            # [nkt, 128, N]
                .transpose(1, 0, 2)             # [128, nkt, N]
                .reshape(KT, nkt * N)
                .astype(dt_np)
            )

            # W shard -> wt layout [n_ch, 128, ch_kt*1024]: chunk c has
            # k-tile g in columns [g*1024:(g+1)*1024], partition p holding
            # contraction row (c*ch_kt+g)*128 + p.
            wt_h = np.zeros((n_ch, KT, cfd), dtype=dt_np)
            if n_real:
                w_stack = rw[rel_ids].reshape(n_real * KT_PER_REL, KT, D)
                full_ch = n_real * KT_PER_REL // ch_kt
                wt_h[:full_ch] = (
                    w_stack[: full_ch * ch_kt]
                    .reshape(full_ch, ch_kt, KT, D)
                    .transpose(0, 2, 1, 3)
                    .reshape(full_ch, KT, cfd)
                    .astype(dt_np)
                )
            in_maps.append({"ht": ht_h, "wt": wt_h})
        batches.append(in_maps)

    return nc, batches, U


def kernel(global_features, speaker, Wq, Wk, rgcn_weight, rgcn_root,
           rgcn_bias, gcn_rel_w, gcn_rel_b, gcn_root_w):
    global LAST_EXEC_TIME_NS
    from concourse.bass_utils import run_bass_kernel_spmd

    nc, batches, _ = _prepare(global_features, speaker, Wq, Wk, rgcn_weight)

    kwargs = {}
    if os.environ.get("BASS_GCN_TRACE"):
        kwargs = dict(trace=True, trace_cores=list(range(N_CORES)))

    out = np.zeros((N, D), dtype=np.float64)
    for in_maps in batches:
        res = run_bass_kernel_spmd(nc, in_maps, list(range(N_CORES)), **kwargs)
        LAST_EXEC_TIME_NS = res.exec_time_ns
        # ---- host all-reduce of the per-core partials ----
        for r in res.results:
            out += r["out"][:N].astype(np.float64)

    # ---- tiny epilogue on host ----
    x = np.asarray(global_features, dtype=np.float64)
    out += x @ np.asarray(rgcn_root, dtype=np.float64)
    out += np.asarray(rgcn_bias, dtype=np.float64)

    # GraphConv, sum aggregation over the dense edge list: every dst sees
    # every src, so agg is the column-sum of `out` broadcast to all rows.
    agg = np.broadcast_to(out.sum(axis=0), (N, D))
    x2 = (
        agg @ np.asarray(gcn_rel_w, dtype=np.float64)
        + np.asarray(gcn_rel_b, dtype=np.float64)
        + out @ np.asarray(gcn_root_w, dtype=np.float64)
    )

    return np.concatenate([x2, x], axis=-1).astype(np.float32)


# revision 22
# speedup vs baseline: 1.0712x; 1.0712x over previous
"""DialogueGCN forward on 8 Trainium2 NeuronCores.

Strategy (expert-parallel over relations, per the sharding hint):
  The dominant cost is streaming the per-edge relation weights
  rgcn_weight[etype] (each [1024,1024]).  The dense 10x10 edge list uses
  only U <= 100 distinct relations out of R=200, so we gather just the
  used slices.  By linearity, the per-edge RGCN messages collapse to

      out = sum_r (A_r @ X) @ W_r        A_r[j,i] = mean-normalized
                                          attention weight of edge i->j
                                          with relation r (else 0)

  i.e. one matmul  [10, U*1024] x [U*1024, 1024].  We shard the U used
  relations across the 8 cores; each core computes its partial [10,1024]
  with a PSUM-accumulated K-loop on the tensor engine, and the partials
  are summed (all-reduce) on the host along with the tiny root/bias and
  second GraphConv layer terms (node features are [10,1024] - negligible).

  Device compute dtype is bf16 (rounded on host): halves the HBM traffic
  of the memory-bound weight stream; end-to-end L2 relative error vs the
  fp32 reference is ~5e-4.
"""

import os

import numpy as np

S = 10
N = 10
D = 1024
R = 2 * S * S
N_CORES = 8
KT = 128              # contraction tile rows (partition dim)
KT_PER_REL = D // KT  # 8 k-tiles per relation

# Filled with the slowest profiled core's HW time (ns) when profiling is
# available (it is not under the axon PJRT path - stays None there).
LAST_EXEC_TIME_NS = None

_NC_CACHE = {}


def _compute_dtype():
    import concourse.mybir as mybir
    import ml_dtypes

    name = os.environ.get("BASS_GCN_DTYPE", "bf16")
    if name == "bf16":
        return mybir.dt.bfloat16, ml_dtypes.bfloat16
    if name == "f32":
        return mybir.dt.float32, np.float32
    if name == "f32r":
        return mybir.dt.float32r, np.float32
    raise ValueError(name)


def _chunk_kt():
    return int(os.environ.get("BASS_GCN_CHUNK_KT", "4"))


def _wide_mm():
    return bool(int(os.environ.get("BASS_GCN_WIDE_MM", "0")))


def _build_nc(n_rel_local, dt_dev):
    """One core's program: out[10,1024] = sum_kt ht_kt.T @ wt_kt.

    Raw bass (no TileContext): the dependency structure is a straight
    pipeline, and Tile's kernel-tail drain needs one sync wait per live
    semaphore, which exceeds the walrus CTRL wait-slot limit once >6 DMA
    lanes are in flight.

    The weight stream is pre-tiled on the host into contiguous chunks of
    ch_kt k-tiles: wt[c] is [128, ch_kt*1024], k-tile g of the chunk in
    columns [g*1024:(g+1)*1024], partition p holding contraction row
    (c*ch_kt + g)*128 + p.  SP streams ht + all chunks on one HWDGE ring
    with a completion semaphore per chunk; PE chases the semaphores,
    accumulating every k-tile into PSUM; DVE copies PSUM to SBUF; SP
    DMAs the partial out.
    """
    import contextlib

    import concourse.bass as bass
    import concourse.mybir as mybir

    nkt = n_rel_local * KT_PER_REL   # total k-tiles
    ch_kt = min(_chunk_kt(), nkt)
    assert nkt % ch_kt == 0
    n_ch = nkt // ch_kt              # DMA chunks
    cfd = ch_kt * D                  # columns of one chunk
    wide = _wide_mm()

    nc = bass.Bass()
    ht = nc.declare_dram_parameter("ht", [KT, nkt * N], dt_dev, isOutput=False)
    wt = nc.declare_dram_parameter("wt", [n_ch, KT, cfd], dt_dev,
                                   isOutput=False)
    # 16 partitions (rows 10..15 are zeros): DMA completion semaphores
    # fire all 16 increments for >=16-partition transfers, but not for 10.
    OP = 16
    out = nc.declare_dram_parameter("out", [OP, D], mybir.dt.float32,
                                    isOutput=True)

    f32 = mybir.dt.float32
    with contextlib.ExitStack() as ctx:
        hsem = ctx.enter_context(nc.semaphore("hsem"))
        wsem = [
            ctx.enter_context(nc.semaphore(f"wsem{c}")) for c in range(n_ch)
        ]
        osem = ctx.enter_context(nc.semaphore("osem"))
        psem = ctx.enter_context(nc.semaphore("psem"))
        vsem = ctx.enter_context(nc.semaphore("vsem"))
        msem = ctx.enter_context(nc.semaphore("msem"))
        hts = ctx.enter_context(nc.sbuf_tensor("hts", [KT, nkt * N], dt_dev))
        ws = ctx.enter_context(nc.sbuf_tensor("ws", [KT, nkt * D], dt_dev))
        if wide:
            acc = ctx.enter_context(nc.psum_tensor("acc", [N, D], f32))
        else:
            acc0 = ctx.enter_context(nc.psum_tensor("acc0", [N, 512], f32))
            acc1 = ctx.enter_context(nc.psum_tensor("acc1", [N, 512], f32))
        o_s = ctx.enter_context(nc.sbuf_tensor("os", [OP, D], f32))

        with nc.Block() as block:

            @block.sync
            def _(sync):
                sync.dma_start(out=hts[:], in_=ht[:]).then_inc(hsem, 16)
                for c in range(n_ch):
                    sync.dma_start(
                        out=ws[:, c * cfd : (c + 1) * cfd], in_=wt[c]
                    ).then_inc(wsem[c], 16)
                sync.wait_ge(vsem, 1)
                sync.dma_start(out=out[:], in_=o_s[:]).then_inc(osem, 16)
                sync.wait_ge(osem, 16)

            @block.tensor
            def _(tensor):
                last_mm = None
                tensor.wait_ge(hsem, 16)
                for c in range(n_ch):
                    tensor.wait_ge(wsem[c], 16)
                    for g in range(ch_kt):
                        kt = c * ch_kt + g
                        lhsT = hts[:, kt * N : (kt + 1) * N]
                        first = kt == 0
                        last = kt == nkt - 1
                        col = kt * D
                        if wide:
                            last_mm = tensor.matmul(
                                acc[:], lhsT, ws[:, col : col + D],
                                start=first, stop=last,
                            )
                        else:
                            tensor.matmul(
                                acc0[:], lhsT, ws[:, col : col + 512],
                                start=first, stop=last,
                            )
                            last_mm = tensor.matmul(
                                acc1[:], lhsT, ws[:, col + 512 : col + D],
                                start=first, stop=last,
                            )
                last_mm.then_inc(psem, 1)

            @block.vector
            def _(vector):
                vector.memset(o_s[:, :], 0).then_inc(msem, 1)
                vector.wait_ge(msem, 1)
                vector.wait_ge(psem, 1)
                if wide:
                    vector.tensor_copy(o_s[0:N, :], acc[:]).then_inc(vsem, 1)
                else:
                    vector.tensor_copy(o_s[0:N, 0:512], acc0[:])
                    vector.tensor_copy(o_s[0:N, 512:1024],
                                       acc1[:]).then_inc(vsem, 1)

    return nc


def _prepare(global_features, speaker, Wq, Wk, rgcn_weight):
    """Host planning: attention, edge weights, per-relation aggregation,
    and the per-core shard arrays.  Returns (nc, batches, U) where
    batches is a list of in_maps (one per SPMD launch)."""
    x = np.asarray(global_features, dtype=np.float64)
    speaker = np.asarray(speaker)

    # ---- attention -> edge weights (tiny, host) ----
    q = x @ np.asarray(Wq, dtype=np.float64)
    k = x @ np.asarray(Wk, dtype=np.float64)
    logits = (q @ k.T) / np.sqrt(np.float64(D))
    logits -= logits.max(axis=-1, keepdims=True)
    attn = np.exp(logits)
    attn /= attn.sum(axis=-1, keepdims=True)

    # ---- dense all-pairs edges, relation ids, per-(rel,dst) mean weights ----
    src = np.repeat(np.arange(N), N)
    dst = np.tile(np.arange(N), N)
    sp = speaker.astype(np.int64)
    etype = 2 * (sp[src] * S + sp[dst]) + (src >= dst).astype(np.int64)
    used, inv = np.unique(etype, return_inverse=True)
    U = len(used)
    seg = etype * N + dst
    cnt = np.bincount(seg, minlength=R * N)
    w_e = attn.reshape(-1) / cnt[seg]

    # ---- per-used-relation aggregated pre-messages H[u,j,:] ----
    H = np.zeros((U, N, D))
    np.add.at(H, (inv, dst), w_e[:, None] * x[src])

    # ---- shard the used relations across cores (batched if U is huge) ----
    max_rel_local = int(os.environ.get("BASS_GCN_MAX_REL_LOCAL", "10"))
    n_rel_local = min(max_rel_local, max(1, -(-U // N_CORES)))
    rel_per_batch = N_CORES * n_rel_local
    n_batch = max(1, -(-U // rel_per_batch))

    dt_dev, dt_np = _compute_dtype()
    nkt = n_rel_local * KT_PER_REL
    ch_kt = min(_chunk_kt(), nkt)
    n_ch = nkt // ch_kt
    cfd = ch_kt * D

    key = (n_rel_local, ch_kt, _wide_mm(), str(dt_dev))
    if key not in _NC_CACHE:
        _NC_CACHE[key] = _build_nc(n_rel_local, dt_dev)
    nc = _NC_CACHE[key]

    rw = np.asarray(rgcn_weight)
    batches = []
    for b in range(n_batch):
        in_maps = []
        for c in range(N_CORES):
            lo = b * rel_per_batch + c * n_rel_local
            rel_ids = used[lo : lo + n_rel_local]
            n_real = len(rel_ids)

            # H shard -> ht layout [128, nkt*10]: k-tile kt occupies columns
            # [kt*10:(kt+1)*10]; partition p holds contraction row kt*128+p.
            h_c = np.zeros((n_rel_local, N, D))
            h_c[:n_real] = H[lo : lo + n_real]
            ht_h = (
                h_c.transpose(0, 2, 1)          # [n_rel, D, N]
                .reshape(nkt, KT, N)            # [nkt, 128, N]
                .transpose(1, 0, 2)             # [128, nkt, N]
                .reshape(KT, nkt * N)
                .astype(dt_np)
            )

            # W shard -> wt layout [n_ch, 128, ch_kt*1024]: chunk c has
            # k-tile g in columns [g*1024:(g+1)*1024], partition p holding
            # contraction row (c*ch_kt+g)*128 + p.
            wt_h = np.zeros((n_ch, KT, cfd), dtype=dt_np)
            if n_real:
                w_stack = rw[rel_ids].reshape(n_real * KT_PER_REL, KT, D)
                full_ch = n_real * KT_PER_REL // ch_kt
                wt_h[:full_ch] = (
                    w_stack[: full_ch * ch_kt]
                    .reshape(full_ch, ch_kt, KT, D)
                    .transpose(0, 2, 1, 3)
                    .reshape(full_ch, KT, cfd)
                    .astype(dt_np)
                )
            in_maps.append({"ht": ht_h, "wt": wt_h})
        batches.append(in_maps)

    return nc, batches, U


def kernel(global_features, speaker, Wq, Wk, rgcn_weight, rgcn_root,
           rgcn_bias, gcn_rel_w, gcn_rel_b, gcn_root_w):
    global LAST_EXEC_TIME_NS
    from concourse.bass_utils import run_bass_kernel_spmd

    nc, batches, _ = _prepare(global_features, speaker, Wq, Wk, rgcn_weight)

    kwargs = {}
    if os.environ.get("BASS_GCN_TRACE"):
        kwargs = dict(trace=True, trace_cores=list(range(N_CORES)))

    out = np.zeros((N, D), dtype=np.float64)
    for in_maps in batches:
        res = run_bass_kernel_spmd(nc, in_maps, list(range(N_CORES)), **kwargs)
        LAST_EXEC_TIME_NS = res.exec_time_ns
        # ---- host all-reduce of the per-core partials ----
        for r in res.results:
            out += r["out"][:N].astype(np.float64)

    # ---- tiny epilogue on host ----
    x = np.asarray(global_features, dtype=np.float64)
    out += x @ np.asarray(rgcn_root, dtype=np.float64)
    out += np.asarray(rgcn_bias, dtype=np.float64)

    # GraphConv, sum aggregation over the dense edge list: every dst sees
    # every src, so agg is the column-sum of `out` broadcast to all rows.
    agg = np.broadcast_to(out.sum(axis=0), (N, D))
    x2 = (
        agg @ np.asarray(gcn_rel_w, dtype=np.float64)
        + np.asarray(gcn_rel_b, dtype=np.float64)
        + out @ np.asarray(gcn_root_w, dtype=np.float64)
    )

    return np.concatenate([x2, x], axis=-1).astype(np.float32)
